# revision 1
# baseline (speedup 1.0000x reference)
"""Trainium2 Bass kernel for nn_Attention_60155311948227 (sparse_attention).

Sharding: data-parallel over batch B=8 across the 8 NeuronCores (1 sample per
core); the four FC weights are replicated (each core DMAs its own copy).

Per-core pipeline (GEMMs in bf16 with fp32 PSUM accumulation):
  XCT  = x_context^T   fp32 HWDGE row-strips -> PE transpose -> bf16
  A^T  = sum_{7x7}(x)  flat-layout loads, DVE reduce, PE transpose
  K^T  = BN(relu(kW @ xc^T + b))   [d1(part), m] bf16, kept in SBUF
  kn2  = ones-matmul of K^T**2 -> rk = 1/||k_row||;  qn2 -> rq
  S    = (Q^T)^T @ K^T  [n, m] * rq (row) * rk (col bcast) + amask, softmax
  P^T  = PE transpose;  P^T rows scaled by rv (V-row norms)
  V^T  -> PE transpose -> V_nat [m(part), d2] bf16 (unnormalized)
  WV^T = V_nat/P^T contraction; F^T = BN(relu(fW @ WV + b)) fp32
  out  = x + F broadcast over 7x7 (flat-layout passes, F via DRAM bounce)

Weights stream as fp32 column-strips on HWDGE and are cast to bf16 on-chip
(ACT/DVE) — the SWDGE cast-DMA path measures only ~45 GB/s aggregate.
"""

import sys

import numpy as np

try:
    import concourse.bacc as bacc
except ImportError:  # pragma: no cover
    sys.path.insert(0, "/opt/trn_rl_repo")
    import concourse.bacc as bacc

import ml_dtypes

import concourse.bass as bass
import concourse.tile as tile
from concourse import mybir
from concourse import bass_utils
from concourse.masks import make_identity

F32 = mybir.dt.float32
BF16 = mybir.dt.bfloat16
AF = mybir.ActivationFunctionType
ALU = mybir.AluOpType
AX = mybir.AxisListType

BN_EPS = 1e-5
NEG_MASK = -50.0
TEMP_INV = 100.0
NORM_EPS = 1e-24

FULL = dict(B=8, n=64, m=2048, D0=1024, C0=2048, D1=2048, D2=2048, KK=49)

P = 128


def build_program(cfg=None, num_devices=8):
    """Emit the SPMD per-core Bass program. Returns the compiled Bacc."""
    cfg = dict(FULL if cfg is None else cfg)
    n, m, D0, C0, D1, D2, KK = (
        cfg["n"], cfg["m"], cfg["D0"], cfg["C0"], cfg["D1"], cfg["D2"], cfg["KK"]
    )
    nc_d0, nc_c0, nc_d1, nc_d2, nc_m = D0 // P, C0 // P, D1 // P, D2 // P, m // P
    n_nt = max(1, m // 512)          # 512-wide moving-dim tiles
    NT = m // n_nt
    inv_kk = 1.0 / KK
    mh = m // 2
    # flat x/out chunking: partition p = (n, dhalf); per-partition contiguous
    DQ = 32                          # D-rows per flat chunk
    FD = DQ * KK                     # flat chunk free size
    NFC = (D0 // 2) // DQ            # number of flat chunks (8)

    nc = bacc.Bacc("TRN2", target_bir_lowering=False, debug=False,
                   num_devices=num_devices)

    def din(name, shape, dt=F32):
        return nc.dram_tensor(name, shape, dt, kind="ExternalInput").ap()

    x_in = din("x", [n, D0, KK])
    xc_in = din("xc", [m, C0])
    wqt = din("wqt", [D0, D1])
    wkt = din("wkt", [C0, D1])
    wvt = din("wvt", [C0, D2])
    wft = din("wft", [D2, D0])
    amask = din("amask", [m], BF16)
    qcb = din("qcb", [P, nc_d1]); qcg = din("qcg", [P, nc_d1]); qc2 = din("qc2", [P, nc_d1])
    kcb = din("kcb", [P, nc_d1]); kcg = din("kcg", [P, nc_d1]); kc2 = din("kc2", [P, nc_d1])
    vcb = din("vcb", [P, nc_d2]); vcg = din("vcg", [P, nc_d2]); vc2 = din("vc2", [P, nc_d2])
    fcb = din("fcb", [P, nc_d0]); fcg = din("fcg", [P, nc_d0]); fc2 = din("fc2", [P, nc_d0])
    out_d = nc.dram_tensor("out", [n, D0, KK], F32, kind="ExternalOutput").ap()
    x_flat = x_in.rearrange("nn d k -> (nn d k)").rearrange(
        "(p f) -> p f", p=P)          # [128, D0*KK/2] per-partition contiguous
    out_flat = out_d.rearrange("nn d k -> (nn d k)").rearrange(
        "(p f) -> p f", p=P)

    with tile.TileContext(nc) as tc:
        with (
            tc.tile_pool(name="consts", bufs=1) as consts,
            tc.tile_pool(name="bigmat", bufs=1) as bigmat,
            tc.tile_pool(name="w8", bufs=2) as w8,          # fp32 strips (8KB)
            tc.tile_pool(name="strips", bufs=3) as strips,  # bf16 strips (4KB)
            tc.tile_pool(name="smalls", bufs=2) as smalls,
            tc.tile_pool(name="wides", bufs=1) as wides,
            tc.tile_pool(name="xpool", bufs=2) as xpool,
            tc.tile_pool(name="ps", bufs=1, space="PSUM") as ps,
            tc.tile_pool(name="dscr", bufs=1, space="DRAM") as dscr,
        ):
            # ---------------- constants ----------------
            ident = consts.tile([P, P], BF16)
            make_identity(nc, ident)
            ident32 = consts.tile([P, P], F32)
            make_identity(nc, ident32)
            ones_col = consts.tile([P, 1], BF16)
            nc.vector.memset(ones_col, 1.0)
            eps_col = consts.tile([P, 1], F32)
            nc.vector.memset(eps_col, NORM_EPS)

            def cload(ap_in, nch):
                t = consts.tile([P, nch], F32, name=f"c_{ap_in.tensor.name}")
                nc.sync.dma_start(out=t, in_=ap_in)
                return t

            qcb_t = cload(qcb, nc_d1); qcg_t = cload(qcg, nc_d1); qc2_t = cload(qc2, nc_d1)
            kcb_t = cload(kcb, nc_d1); kcg_t = cload(kcg, nc_d1); kc2_t = cload(kc2, nc_d1)
            vcb_t = cload(vcb, nc_d2); vcg_t = cload(vcg, nc_d2); vc2_t = cload(vc2, nc_d2)
            fcb_t = cload(fcb, nc_d0); fcg_t = cload(fcg, nc_d0); fc2_t = cload(fc2, nc_d0)

            amask_bc = consts.tile([n, m], BF16, tag="amask_bc")
            nc.gpsimd.dma_start(
                out=amask_bc,
                in_=bass.AP(tensor=amask.tensor, offset=amask.offset,
                            ap=[[0, n]] + list(amask.ap)),
            )

            # ---------------- XCT: transpose x_context ----------------
            # contiguous fp32 row-strips; 16 fp32 PE transposes per strip into
            # an 8KB PSUM tile (alternating tag A/B); ACT copy casts to bf16.
            xct = bigmat.tile([P, nc_c0, m], BF16, tag="xct")
            for i in range(nc_m):
                xcs = w8.tile([P, C0], F32, tag="w8", name="xcs")
                nc.sync.dma_start(out=xcs, in_=xc_in[i * P:(i + 1) * P, :])
                tpx = ps.tile([P, nc_c0, P], F32,
                              tag=("A" if i % 2 == 0 else "B"), name="tpx")
                for c in range(nc_c0):
                    nc.tensor.transpose(tpx[:, c, :], xcs[:, c * P:(c + 1) * P],
                                        ident32)
                nc.scalar.copy(out=xct[:, :, i * P:(i + 1) * P], in_=tpx)

            # ---------------- pooling: A^T = sum_k x (flat layout) ----------
            at = consts.tile([P, nc_d0, n], BF16)
            for g in range(NFC):
                xt = xpool.tile([P, DQ, KK], F32, tag="x", name="xt")
                nc.sync.dma_start(out=xt,
                                  in_=x_flat[:, g * FD:(g + 1) * FD])
                asum = smalls.tile([P, DQ], F32, name="asum")
                nc.vector.reduce_sum(asum, xt, axis=AX.X)
                atp = ps.tile([DQ, P], F32, tag="B", name="atp")
                nc.tensor.transpose(atp, asum, ident32)
                # columns p=(nn, dhalf); D row = dhalf*D0/2 + g*DQ + dd2
                for half in range(2):
                    dglob = half * (D0 // 2) + g * DQ
                    base = dglob % P
                    nc.vector.tensor_copy(
                        out=at[base:base + DQ, dglob // P, :],
                        in_=atp[:, half::2])

            # ---------------- K^T projection (kept in SBUF) ----------------
            def wstrip(w_ap, j, ncc, name):
                """column-strip [P, ncc, P] bf16: even j via sync HWDGE fp32 +
                engine cast; odd j via SWDGE cast-DMA (parallel channel)."""
                wb = strips.tile([P, ncc, P], BF16, tag="strip", name=f"{name}b")
                src_ap = w_ap[:, j * P:(j + 1) * P].rearrange(
                    "(c p) w -> p c w", p=P)
                if j % 2 == 1:
                    nc.gpsimd.dma_start(out=wb, in_=src_ap)
                    return wb
                wf = w8.tile([P, ncc, P], F32, tag="w8", name=f"{name}f")
                nc.sync.dma_start(out=wf, in_=src_ap)
                nc.vector.tensor_copy(out=wb, in_=wf)
                return wb

            kt = bigmat.tile([P, nc_d1, m], BF16, tag="ktv", name="kt")
            kn2 = ps.tile([1, m], F32, tag="B")
            for j in range(nc_d1):
                kws = wstrip(wkt, j, nc_c0, "kws")
                kp = ps.tile([P, m], F32, tag="A", name="kp")
                for c in range(nc_c0):
                    for nt in range(n_nt):
                        nc.tensor.matmul(kp[:, nt * NT:(nt + 1) * NT],
                                         kws[:, c, :],
                                         xct[:, c, nt * NT:(nt + 1) * NT],
                                         start=(c == 0), stop=(c == nc_c0 - 1))
                ktj = kt[:, j, :]
                nc.scalar.activation(ktj[:, :mh], kp[:, :mh], AF.Relu,
                                     bias=kcb_t[:, j:j + 1])
                nc.vector.tensor_scalar(out=ktj[:, mh:], in0=kp[:, mh:],
                                        scalar1=kcb_t[:, j:j + 1], scalar2=0.0,
                                        op0=ALU.add, op1=ALU.max)
                nc.vector.tensor_scalar(out=ktj, in0=ktj,
                                        scalar1=kcg_t[:, j:j + 1],
                                        scalar2=kc2_t[:, j:j + 1],
                                        op0=ALU.mult, op1=ALU.add)
                ksq = w8.tile([P, m], BF16, tag="w8", name="ksq")
                nc.vector.tensor_mul(ksq, ktj, ktj)
                for nt in range(n_nt):
                    nc.tensor.matmul(kn2[:, nt * NT:(nt + 1) * NT], ones_col,
                                     ksq[:, nt * NT:(nt + 1) * NT],
                                     start=(j == 0), stop=(j == nc_d1 - 1))
            # rk chain: sqrt -> scatter [P, m/P] -> recip -> DRAM -> bcast
            rk_row = smalls.tile([1, m], F32, name="rk_row")
            nc.scalar.activation(rk_row, kn2, AF.Sqrt, bias=eps_col[:1, :])
            scr_k = dscr.tile([m], F32, name="scr_k")
            nc.gpsimd.dma_start(out=scr_k, in_=rk_row)
            rk128 = smalls.tile([P, nc_m], F32, name="rk128")
            nc.gpsimd.dma_start(out=rk128,
                                in_=bass.AP(tensor=scr_k.tensor, offset=scr_k.offset,
                                            ap=[[1, P], [P, nc_m]]))
            nc.vector.reciprocal(rk128, rk128)
            scr_k2 = dscr.tile([m], F32, name="scr_k2")
            nc.gpsimd.dma_start(
                out=bass.AP(tensor=scr_k2.tensor, offset=scr_k2.offset,
                            ap=[[1, P], [P, nc_m]]),
                in_=rk128)
            rk_bc = wides.tile([n, m], F32, name="rk_bc", tag="rk_bc")
            nc.gpsimd.dma_start(out=rk_bc,
                                in_=bass.AP(tensor=scr_k2.tensor, offset=scr_k2.offset,
                                            ap=[[0, n], [1, m]]))

            # ---------------- Q^T projection (c-incremental) ----------------
            # contiguous qwt row-strips; all 16 j-blocks accumulate in one
            # [P, nc_d1, n] PSUM tile across the 8 contraction chunks.
            qt = consts.tile([P, nc_d1, n], BF16)
            qps = ps.tile([P, nc_d1, n], F32, tag="B", name="qps")
            for c in range(nc_d0):
                qw8 = w8.tile([P, D1], F32, tag="w8", name="qw8")
                nc.sync.dma_start(out=qw8, in_=wqt[c * P:(c + 1) * P, :])
                qwb = strips.tile([P, D1], BF16, tag="strip", name="qwb")
                if c % 2 == 0:
                    nc.vector.tensor_copy(out=qwb, in_=qw8)
                else:
                    nc.scalar.copy(out=qwb, in_=qw8)
                for j in range(nc_d1):
                    # zero region = 2KB: j-blocks of n*4B; start only on the
                    # first matmul touching each region
                    jperz = max(1, 512 // n)
                    nc.tensor.matmul(qps[:, j, :], qwb[:, j * P:(j + 1) * P],
                                     at[:, c, :],
                                     start=(c == 0 and j % jperz == 0),
                                     stop=(c == nc_d0 - 1 and
                                           j % jperz == jperz - 1),
                                     skip_group_check=True)
            qn2 = ps.tile([1, n], F32, tag="A")
            for j in range(nc_d1):
                q1 = smalls.tile([P, n], BF16, name="q1")
                nc.scalar.activation(q1, qps[:, j, :], AF.Relu,
                                     bias=qcb_t[:, j:j + 1], scale=inv_kk)
                nc.vector.tensor_scalar(out=qt[:, j, :], in0=q1,
                                        scalar1=qcg_t[:, j:j + 1],
                                        scalar2=qc2_t[:, j:j + 1],
                                        op0=ALU.mult, op1=ALU.add)
                qsq = smalls.tile([P, n], BF16, name="qsq")
                nc.scalar.activation(qsq, qt[:, j, :], AF.Square)
                nc.tensor.matmul(qn2, ones_col, qsq,
                                 start=(j == 0), stop=(j == nc_d1 - 1))
            rq_row = smalls.tile([1, n], F32, name="rq_row")
            nc.scalar.activation(rq_row, qn2, AF.Sqrt, bias=eps_col[:1, :])
            scr_q = dscr.tile([n], F32, name="scr_q")
            nc.gpsimd.dma_start(out=scr_q, in_=rq_row)
            rq_col = smalls.tile([n, 1], F32, name="rq_col")
            nc.gpsimd.dma_start(out=rq_col,
                                in_=bass.AP(tensor=scr_q.tensor, offset=scr_q.offset,
                                            ap=[[1, n], [1, 1]]))
            nc.vector.reciprocal(rq_col, rq_col)

            # ---------------- S = Q K^T, softmax ----------------
            sp = ps.tile([n, m], F32, tag="B", name="sp")
            for j in range(nc_d1):
                for nt in range(n_nt):
                    nc.tensor.matmul(sp[:, nt * NT:(nt + 1) * NT], qt[:, j, :],
                                     kt[:, j, nt * NT:(nt + 1) * NT],
                                     start=(j == 0), stop=(j == nc_d1 - 1))
            nc.vector.tensor_scalar(out=sp, in0=sp, scalar1=rq_col,
                                    scalar2=None, op0=ALU.mult)
            nc.vector.tensor_mul(sp, sp, rk_bc)
            nc.vector.tensor_add(sp, sp, amask_bc)
            mxn = smalls.tile([n, 1], F32, name="mxn")
            nc.vector.tensor_reduce(mxn, sp, axis=AX.X, op=ALU.max, negate=True)
            ebias = smalls.tile([n, 1], F32, name="ebias")
            nc.vector.tensor_scalar_mul(ebias, mxn, TEMP_INV)
            p_t = consts.tile([n, m], BF16, name="p_t", tag="amask_bc")
            pden = smalls.tile([n, 1], F32, name="pden")
            nc.scalar.activation(p_t, sp, AF.Exp, bias=ebias, scale=TEMP_INV,
                                 accum_out=pden)
            nc.vector.reciprocal(pden, pden)
            nc.vector.tensor_scalar_mul(p_t, p_t, pden)
            ptp = ps.tile([P, nc_m, n], BF16, tag="B", name="ptp")
            for i in range(nc_m):
                nc.tensor.transpose(ptp[:, i, :], p_t[:, i * P:(i + 1) * P],
                                    ident[:n, :n])
            pt_sb = consts.tile([P, nc_m, n], BF16)
            nc.vector.tensor_copy(out=pt_sb, in_=ptp)

            # ---------------- V^T -> V_nat (unnormalized) ----------------
            v_nat = bigmat.tile([P, nc_m, D2], BF16, tag="ktv", name="v_nat")
            for j in range(nc_d2):
                vws = wstrip(wvt, j, nc_c0, "vws")
                vp = ps.tile([P, m], F32, tag="A", name="vp")
                for c in range(nc_c0):
                    for nt in range(n_nt):
                        nc.tensor.matmul(vp[:, nt * NT:(nt + 1) * NT],
                                         vws[:, c, :],
                                         xct[:, c, nt * NT:(nt + 1) * NT],
                                         start=(c == 0), stop=(c == nc_c0 - 1))
                vtj = strips.tile([P, m], BF16, tag="strip", name="vtj")
                nc.scalar.activation(vtj[:, :mh], vp[:, :mh], AF.Relu,
                                     bias=vcb_t[:, j:j + 1])
                nc.vector.tensor_scalar(out=vtj[:, mh:], in0=vp[:, mh:],
                                        scalar1=vcb_t[:, j:j + 1], scalar2=0.0,
                                        op0=ALU.add, op1=ALU.max)
                nc.vector.tensor_scalar(out=vtj, in0=vtj,
                                        scalar1=vcg_t[:, j:j + 1],
                                        scalar2=vc2_t[:, j:j + 1],
                                        op0=ALU.mult, op1=ALU.add)
                vtp = ps.tile([P, nc_m, P], BF16, tag="B", name="vtp")
                for i in range(nc_m):
                    nc.tensor.transpose(vtp[:, i, :], vtj[:, i * P:(i + 1) * P],
                                        ident)
                nc.vector.tensor_copy(out=v_nat[:, :, j * P:(j + 1) * P],
                                      in_=vtp)
            # rv = 1/||v_row||; folded into P^T rows (per-partition there)
            for i in range(nc_m):
                vsq = w8.tile([P, D2], BF16, tag="w8", name="vsq")
                vn2 = smalls.tile([P, 1], F32, name="vn2")
                nc.scalar.activation(vsq, v_nat[:, i, :], AF.Square,
                                     accum_out=vn2)
                rv = smalls.tile([P, 1], F32, name="rv")
                nc.scalar.activation(rv, vn2, AF.Sqrt, bias=eps_col)
                nc.vector.reciprocal(rv, rv)
                nc.vector.tensor_scalar_mul(pt_sb[:, i, :], pt_sb[:, i, :], rv)

            # ------------- WV^T and F^T fused over d2 chunks -------------
            # per d2-chunk j: WV_j = sum_i V_nat_i^T P^T_i, then immediately
            # accumulated into F via the j-th fwt row-strip (contiguous load).
            fps = ps.tile([P, nc_d0, n], F32, tag="B", name="fps")
            for j in range(nc_d2):
                fw8 = w8.tile([P, D0], F32, tag="w8", name="fw8")
                nc.sync.dma_start(out=fw8, in_=wft[j * P:(j + 1) * P, :])
                fwb = strips.tile([P, D0], BF16, tag="strip", name="fwb")
                if j % 2 == 0:
                    nc.vector.tensor_copy(out=fwb, in_=fw8)
                else:
                    nc.scalar.copy(out=fwb, in_=fw8)
                wvp = ps.tile([P, n], F32, tag="A", name="wvp")
                for i in range(nc_m):
                    nc.tensor.matmul(wvp, v_nat[:, i, j * P:(j + 1) * P],
                                     pt_sb[:, i, :],
                                     start=(i == 0), stop=(i == nc_m - 1))
                wvj = smalls.tile([P, n], BF16, name="wvj")
                nc.vector.tensor_copy(out=wvj, in_=wvp)
                ddperz = max(1, 512 // n)
                for dd in range(nc_d0):
                    nc.tensor.matmul(fps[:, dd, :], fwb[:, dd * P:(dd + 1) * P],
                                     wvj,
                                     start=(j == 0 and dd % ddperz == 0),
                                     stop=(j == nc_d2 - 1 and
                                           dd % ddperz == ddperz - 1),
                                     skip_group_check=True)
            ft = consts.tile([P, nc_d0, n], F32)
            for dd in range(nc_d0):
                f1 = smalls.tile([P, n], F32, name="f1")
                nc.scalar.activation(f1, fps[:, dd, :], AF.Relu,
                                     bias=fcb_t[:, dd:dd + 1])
                nc.vector.tensor_scalar(out=ft[:, dd, :], in0=f1,
                                        scalar1=fcg_t[:, dd:dd + 1],
                                        scalar2=fc2_t[:, dd:dd + 1],
                                        op0=ALU.mult, op1=ALU.add)

            # ---------------- out = x + F (flat layout) ----------------
            # F^T -> F_nat (PE transposes) -> DRAM bounce -> [(n dhalf), D0/2]
            fnat = wides.tile([n, D0], F32, tag="rk_bc")
            for dd in range(nc_d0):
                ftp = ps.tile([n, P], F32, tag="B", name="ftp")
                nc.tensor.transpose(ftp, ft[:, dd, :], ident32)
                nc.vector.tensor_copy(out=fnat[:, dd * P:(dd + 1) * P], in_=ftp)
            f_scr = dscr.tile([n, D0], F32, name="f_scr")
            nc.sync.dma_start(out=f_scr, in_=fnat)
            fperm = wides.tile([P, D0 // 2], F32, name="fperm", tag="rk_bc")
            nc.sync.dma_start(
                out=fperm,
                in_=bass.AP(tensor=f_scr.tensor, offset=f_scr.offset,
                            ap=[[D0, n], [D0 // 2, 2], [1, D0 // 2]]))
            for g in range(NFC):
                xo = xpool.tile([P, DQ, KK], F32, tag="x", name="xo")
                nc.sync.dma_start(out=xo, in_=x_flat[:, g * FD:(g + 1) * FD])
                nc.vector.tensor_add(
                    xo, xo,
                    fperm[:, g * DQ:(g + 1) * DQ].unsqueeze(2)
                    .broadcast_to([P, DQ, KK]))
                nc.scalar.dma_start(out=out_flat[:, g * FD:(g + 1) * FD], in_=xo)

    nc.compile()
    return nc


_CACHED = {}
# test-harness hook: extra kwargs for run_bass_kernel_spmd (e.g. trace=True)
_RUN_KWARGS = {}


def _get_program():
    if "nc" not in _CACHED:
        _CACHED["nc"] = build_program()
    return _CACHED["nc"]


def _bn_consts(b, gamma, beta, mean, var, nch):
    g = (gamma / np.sqrt(var + BN_EPS)).astype(np.float32)
    b2 = (beta - g * mean).astype(np.float32)
    def fold(v):
        return np.ascontiguousarray(np.asarray(v, np.float32).reshape(nch, P).T)
    return fold(b), fold(g), fold(b2)


def kernel(**inputs):
    cfg = FULL
    B, n, m = cfg["B"], cfg["n"], cfg["m"]
    D0, C0, D1, D2, KK = cfg["D0"], cfg["C0"], cfg["D1"], cfg["D2"], cfg["KK"]

    x = np.asarray(inputs["x"], dtype=np.float32).reshape(B, n, D0, KK)
    xc = np.asarray(inputs["x_context"], dtype=np.float32)
    nvalid = np.asarray(inputs["num_valid_context_items"]).reshape(B).astype(np.int64)

    wqt = np.ascontiguousarray(np.asarray(inputs["q_W"], np.float32).T)
    wkt = np.ascontiguousarray(np.asarray(inputs["k_W"], np.float32).T)
    wvt = np.ascontiguousarray(np.asarray(inputs["v_W"], np.float32).T)
    wft = np.ascontiguousarray(np.asarray(inputs["f_W"], np.float32).T)

    qc = _bn_consts(inputs["q_b"], inputs["q_gamma"], inputs["q_beta"],
                    inputs["q_mean"], inputs["q_var"], D1 // P)
    kc = _bn_consts(inputs["k_b"], inputs["k_gamma"], inputs["k_beta"],
                    inputs["k_mean"], inputs["k_var"], D1 // P)
    vc = _bn_consts(inputs["v_b"], inputs["v_gamma"], inputs["v_beta"],
                    inputs["v_mean"], inputs["v_var"], D2 // P)
    fc = _bn_consts(inputs["f_b"], inputs["f_gamma"], inputs["f_beta"],
                    inputs["f_mean"], inputs["f_var"], D0 // P)

    ar = np.arange(m)
    in_maps = []
    for b in range(B):
        am = np.where(ar < nvalid[b], 0.0, NEG_MASK).astype(ml_dtypes.bfloat16)
        in_maps.append({
            "x": np.ascontiguousarray(x[b]),
            "xc": np.ascontiguousarray(xc[b]),
            "wqt": wqt, "wkt": wkt, "wvt": wvt, "wft": wft,
            "amask": am,
            "qcb": qc[0], "qcg": qc[1], "qc2": qc[2],
            "kcb": kc[0], "kcg": kc[1], "kc2": kc[2],
            "vcb": vc[0], "vcg": vc[1], "vc2": vc[2],
            "fcb": fc[0], "fcg": fc[1], "fc2": fc[2],
        })

    nc = _get_program()
    res = bass_utils.run_bass_kernel_spmd(nc, in_maps, core_ids=list(range(B)),
                                          **_RUN_KWARGS)
    _CACHED["last_results"] = res
    out = np.stack([r["out"] for r in res.results], axis=0)
    return out.reshape(B, n, D0, 7, 7).astype(np.float32)



# revision 9
# speedup vs baseline: 1.6485x; 1.6485x over previous
"""Trainium2 Bass kernel for nn_Attention_60155311948227 (sparse_attention).

Data-parallel over batch B=8 across 8 NeuronCores (1 sample/core). Host-side
(numpy) prep: all weights pre-cast to bf16 in matmul-ready layouts with the
BatchNorm gamma/rsqrt(var) folded into the weight columns; x_context
pre-transposed to xc^T bf16; context length trimmed to mp =
roundup(max(num_valid), 256) (invalid tail masked on-chip as before).

Per-core pipeline (bf16 GEMMs, fp32 PSUM, 4-slot PSUM rotation so the PE
stream never stalls on the trailing BN work):
  K^T  = relu(kW' @ xc^T + b') + c'   [d1(part), mp] bf16, kept in SBUF
  kn2  via DVE square + Pool-engine accumulate, ones-matmul column sum
  A^T  = sum_{7x7}(x) (DVE reduces, PE transposes)
  Q    = natural [n, d1] (stationary = A^T chunks), BN via broadcast tiles,
         L2-normalized rows (rq folded in), then PE-transposed for S
  S    = [n, mp] in two halves; softmax unnormalized (denominator folded
         into WV rows later); P^T via PE transposes
  V    = natural [m(part), d2] directly (stationary = xc^T chunks) -- no
         V^T transposes; BN via broadcast tiles; row norms folded into P^T
  WV   = [n, d2] natural (stationary = P^T chunks)
  F    = [n, d0] natural (stationary = WV^T chunks), BN via broadcast tiles
  out  = x + F broadcast over 7x7 (flat layout, F via DRAM bounce)
"""

import sys

import numpy as np

try:
    import concourse.bacc as bacc
except ImportError:  # pragma: no cover
    sys.path.insert(0, "/opt/trn_rl_repo")
    import concourse.bacc as bacc

import ml_dtypes

import concourse.bass as bass
import concourse.tile as tile
from concourse import mybir
from concourse import bass_utils
from concourse.masks import make_identity

F32 = mybir.dt.float32
BF16 = mybir.dt.bfloat16
AF = mybir.ActivationFunctionType
ALU = mybir.AluOpType
AX = mybir.AxisListType

BN_EPS = 1e-5
NEG_MASK = -50.0
TEMP_INV = 100.0
NORM_EPS = 1e-24

B, N, M, D0, C0, D1, D2, KK = 8, 64, 2048, 1024, 2048, 2048, 2048, 49
P = 128

# flat x layout: partition p = (nn, dhalf); 16 chunks of DQ=32 D-rows
DQ = 32
FD = DQ * KK            # 1568 floats per chunk per partition
NFC = (D0 // 2) // DQ   # 16 chunks


def _mtiles(width):
    """512-aligned PSUM accumulation tiles covering [0, width)."""
    return [(s, min(512, width - s)) for s in range(0, width, 512)]


def build_program(mp, num_devices=8):
    """Emit the SPMD per-core Bass program for context length mp."""
    assert mp % 256 == 0 and 512 <= mp <= M
    mh = mp // 2                 # m-half (PSUM-sized S/K tiles)
    nmc = mp // P                # m-chunks for V/P^T
    nc_c0, nc_d1, nc_d2, nc_d0 = C0 // P, D1 // P, D2 // P, D0 // P

    nc = bacc.Bacc("TRN2", target_bir_lowering=False, debug=False,
                   num_devices=num_devices)

    def din(name, shape, dt=BF16):
        return nc.dram_tensor(name, shape, dt, kind="ExternalInput").ap()

    x_in = din("x", [N, D0, KK], F32)
    xct_d = din("xct", [C0, mp])
    wk_d = din("wk", [nc_d1, P, nc_c0, P])       # (j, p=c-part, c, q=d1-col)
    wq_d = din("wq", [nc_d0, P, D1])        # (c, p=d0-part, d1)
    wv_d = din("wv", [8, P, nc_c0, 256])         # (eighth, p=c-part, c, d2)
    wf_d = din("wf", [nc_d2, P, D0])             # (c, p=d2-part, d0)
    kcb_d = din("kcb", [P, nc_d1], F32)
    kcc_d = din("kcc", [P, nc_d1], F32)
    qbv = din("qbv", [D1]); qcv = din("qcv", [D1])
    vbv = din("vbv", [D2]); vcv = din("vcv", [D2])
    fbv = din("fbv", [D0]); fcv = din("fcv", [D0])
    amask = din("amask", [mp])
    out_d = nc.dram_tensor("out", [N, D0, KK], F32, kind="ExternalOutput").ap()

    x_flat = x_in.rearrange("nn d k -> (nn d k)").rearrange(
        "(p f) -> p f", p=P)
    out_flat = out_d.rearrange("nn d k -> (nn d k)").rearrange(
        "(p f) -> p f", p=P)

    with tile.TileContext(nc) as tc:
        with (
            tc.tile_pool(name="consts", bufs=1) as consts,
            tc.tile_pool(name="bigmat", bufs=1) as bigmat,
            tc.tile_pool(name="strips", bufs=2) as strips,
            tc.tile_pool(name="wvq", bufs=2) as wvqp,
            tc.tile_pool(name="bc", bufs=1) as bcp,
            tc.tile_pool(name="nats", bufs=1) as nats,
            tc.tile_pool(name="sq", bufs=1) as sqp,
            tc.tile_pool(name="smalls", bufs=2) as smalls,
            tc.tile_pool(name="xpool", bufs=2) as xpool,
            tc.tile_pool(name="ps", bufs=1, space="PSUM") as ps,
            tc.tile_pool(name="dscr", bufs=1, space="DRAM") as dscr,
        ):
            # ---- PSUM 4-slot rotation (4 x 4KB) ----
            _slot = [0]

            def pnext(shape, dtype, name):
                t = ps.tile(shape, dtype, tag="ABCD"[_slot[0] % 4], name=name)
                _slot[0] += 1
                return t

            # ---------------- constants ----------------
            ident = consts.tile([P, P], BF16)
            make_identity(nc, ident)
            ident32 = consts.tile([P, P], F32)
            make_identity(nc, ident32)
            ones_bf = consts.tile([P, 1], BF16)
            nc.vector.memset(ones_bf, 1.0)
            eps1 = consts.tile([1, 1], F32)
            nc.vector.memset(eps1, NORM_EPS)
            epsc = consts.tile([P, 1], F32)
            nc.vector.memset(epsc, NORM_EPS)
            kcb_t = consts.tile([P, nc_d1], F32)
            nc.sync.dma_start(out=kcb_t, in_=kcb_d)
            kcc_t = consts.tile([P, nc_d1], F32)
            nc.sync.dma_start(out=kcc_t, in_=kcc_d)

            def bcast(vec, rows, nch, tag, name):
                t = bcp.tile([rows, nch], BF16, tag=tag, name=name)
                nc.gpsimd.dma_start(
                    out=t, in_=bass.AP(tensor=vec.tensor, offset=vec.offset,
                                       ap=[[0, rows]] + list(vec.ap)))
                return t

            amask_bc = consts.tile([N, mp], BF16, name="amask_bc")
            nc.gpsimd.dma_start(
                out=amask_bc,
                in_=bass.AP(tensor=amask.tensor, offset=amask.offset,
                            ap=[[0, N]] + list(amask.ap)))

            # ---------------- big SBUF tensors ----------------
            xct = bigmat.tile([P, nc_c0, mp], BF16, tag="xct")
            for c4 in range(4):
                nc.scalar.dma_start(
                    out=xct[:, c4 * 4:(c4 + 1) * 4, :],
                    in_=xct_d[c4 * 512:(c4 + 1) * 512, :].rearrange(
                        "(c p) m -> p c m", p=P))
            kt = bigmat.tile([P, nc_d1, mp], BF16, tag="ktv", name="kt")
            k2a = consts.tile([P, mp], BF16, name="k2a")

            # x chunks for pooling (DVE reduces into asums)
            at = consts.tile([P, nc_d0, N], BF16, name="at")
            asums = consts.tile([P, NFC, DQ], F32, name="asums")
            for g in range(NFC):
                xt = xpool.tile([P, DQ, KK], F32, tag="x", name="xt")
                nc.scalar.dma_start(out=xt, in_=x_flat[:, g * FD:(g + 1) * FD])
                nc.vector.tensor_reduce(asums[:, g, :], xt, axis=AX.X,
                                        op=ALU.add)

            # ---------------- K^T projection ----------------
            for j in range(nc_d1):
                kws = strips.tile([P, nc_c0, P], BF16, tag="strip",
                                  name="kws")
                nc.sync.dma_start(out=kws, in_=wk_d[j])
                for h in range(2):
                    kp = pnext([P, mh], F32, "kp")
                    for c in range(nc_c0):
                        for (s, w) in _mtiles(mh):
                            nc.tensor.matmul(
                                kp[:, s:s + w], kws[:, c, :],
                                xct[:, c, h * mh + s:h * mh + s + w],
                                start=(c == 0), stop=(c == nc_c0 - 1))
                    ktj = kt[:, j, h * mh:(h + 1) * mh]
                    sp0 = min(512, mh)
                    nc.scalar.activation(ktj[:, :sp0], kp[:, :sp0], AF.Relu,
                                         bias=kcb_t[:, j:j + 1])
                    if mh > sp0:
                        nc.vector.tensor_scalar(
                            out=ktj[:, sp0:], in0=kp[:, sp0:],
                            scalar1=kcb_t[:, j:j + 1], scalar2=0.0,
                            op0=ALU.add, op1=ALU.max)
                    nc.vector.tensor_scalar(
                        out=ktj, in0=ktj, scalar1=kcc_t[:, j:j + 1],
                        scalar2=None, op0=ALU.add)
                    ksq = sqp.tile([P, mh], BF16, tag="sq", name="ksq")
                    nc.vector.tensor_mul(ksq, ktj, ktj)
                    dst = k2a[:, h * mh:(h + 1) * mh]
                    with nc.allow_low_precision(reason="k row-norm accum; "
                                                "2e-2 rel tolerance"):
                        if j == 0:
                            nc.gpsimd.tensor_copy(out=dst, in_=ksq)
                        else:
                            nc.gpsimd.tensor_add(dst, dst, ksq)

            # ---------------- pooling transposes -> A^T ----------------
            for gq in range(NFC // 8):
                atp = pnext([DQ, 8, P], F32, "atp")
                for g8 in range(8):
                    g = gq * 8 + g8
                    nc.tensor.transpose(atp[:, g8, :], asums[:, g, :],
                                        ident32)
                for g8 in range(8):
                    g = gq * 8 + g8
                    for half in range(2):
                        dglob = half * (D0 // 2) + g * DQ
                        base = dglob % P
                        nc.vector.tensor_copy(
                            out=at[base:base + DQ, dglob // P, :],
                            in_=atp[:, g8, half::2])

            # ---------------- kn2 -> rk ----------------
            rk_scr = dscr.tile([mp], BF16, name="rk_scr")
            for h in range(2):
                kn2 = pnext([1, mh], F32, "kn2")
                for (s, w) in _mtiles(mh):
                    nc.tensor.matmul(kn2[:, s:s + w], ones_bf,
                                     k2a[:, h * mh + s:h * mh + s + w],
                                     start=True, stop=True)
                rkh = sqp.tile([1, mh], F32, tag="sq", name="rkh")
                nc.scalar.activation(rkh, kn2, AF.Sqrt, bias=eps1)
                rkb = sqp.tile([1, mh], BF16, tag="sq2", name="rkb")
                with nc.allow_low_precision(reason="rk bf16; 2e-2 tolerance"):
                    nc.vector.reciprocal(rkb, rkh)
                nc.gpsimd.dma_start(out=rk_scr[h * mh:(h + 1) * mh], in_=rkb)
            rk_bc = consts.tile([N, mp], BF16, name="rk_bc")
            nc.gpsimd.dma_start(
                out=rk_bc, in_=bass.AP(tensor=rk_scr.tensor,
                                       offset=rk_scr.offset,
                                       ap=[[0, N], [1, mp]]))

            # ---------------- Q natural [n, d1] ----------------
            qb_bc = bcast(qbv, N, D1, "b", "qb_bc")
            qc_bc = bcast(qcv, N, D1, "c", "qc_bc")
            qps = [pnext([N, 1024], F32, "qpA"), pnext([N, 1024], F32, "qpB")]
            for c in range(nc_d0):
                qw = strips.tile([P, D1], BF16, tag="strip", name="qw")
                eng = nc.sync if c % 2 == 0 else nc.scalar
                eng.dma_start(out=qw, in_=wq_d[c])
                for h in range(2):
                    for (s, w) in _mtiles(1024):
                        nc.tensor.matmul(qps[h][:, s:s + w], at[:, c, :],
                                         qw[:, h * 1024 + s:h * 1024 + s + w],
                                         start=(c == 0),
                                         stop=(c == nc_d0 - 1))
            q_nat = nats.tile([N, D1], BF16, tag="nat2", name="q_nat")
            qn2 = smalls.tile([N, 1], F32, name="qn2")
            for h in range(2):
                qh = q_nat[:, h * 1024:(h + 1) * 1024]
                nc.vector.tensor_add(qh, qps[h],
                                     qb_bc[:, h * 1024:(h + 1) * 1024])
                nc.vector.tensor_scalar_max(qh, qh, 0.0)
                nc.vector.tensor_add(qh, qh,
                                     qc_bc[:, h * 1024:(h + 1) * 1024])
                qsq = sqp.tile([N, 1024], BF16, tag="sq", name="qsq")
                qn2h = smalls.tile([N, 1], F32, name="qn2h")
                nc.scalar.activation(qsq, qh, AF.Square, accum_out=qn2h)
                if h == 0:
                    nc.vector.tensor_copy(out=qn2, in_=qn2h)
                else:
                    nc.vector.tensor_add(qn2, qn2, qn2h)
            rq = smalls.tile([N, 1], F32, name="rq")
            nc.scalar.activation(rq, qn2, AF.Sqrt, bias=epsc[:N, :])
            nc.vector.reciprocal(rq, rq)
            nc.vector.tensor_scalar(out=q_nat, in0=q_nat, scalar1=rq,
                                    scalar2=None, op0=ALU.mult)
            qt_ps = pnext([P, nc_d1, N], BF16, "qt_ps")
            for c in range(nc_d1):
                nc.tensor.transpose(qt_ps[:, c, :],
                                    q_nat[:, c * P:(c + 1) * P], ident[:N, :N])
            qt_sb = consts.tile([P, nc_d1, N], BF16, name="qt_sb")
            nc.vector.tensor_copy(out=qt_sb, in_=qt_ps)

            # ---------------- S = Q K^T (two halves) + softmax ----------
            vb_bc = bcast(vbv, P, D2, "vb", "vb_bc")   # early for V BN
            vc_bc = bcast(vcv, P, D2, "vc", "vc_bc")
            p_t = consts.tile([N, mp], BF16, name="p_t")
            mx = smalls.tile([N, 1], F32, name="mx")
            pden = smalls.tile([N, 1], F32, name="pden")
            sps = []
            for h in range(2):
                sph = pnext([N, mh], F32, "sph")
                sps.append(sph)
                for j in range(nc_d1):
                    for (s, w) in _mtiles(mh):
                        nc.tensor.matmul(sph[:, s:s + w], qt_sb[:, j, :],
                                         kt[:, j, h * mh + s:h * mh + s + w],
                                         start=(j == 0), stop=(j == nc_d1 - 1))
                nc.vector.tensor_mul(sph, sph, rk_bc[:, h * mh:(h + 1) * mh])
                nc.vector.tensor_add(sph, sph,
                                     amask_bc[:, h * mh:(h + 1) * mh])
                mxh = smalls.tile([N, 1], F32, name="mxh")
                nc.vector.tensor_reduce(mxh, sph, axis=AX.X, op=ALU.max)
                if h == 0:
                    nc.vector.tensor_copy(out=mx, in_=mxh)
                else:
                    nc.vector.tensor_max(mx, mx, mxh)
            ebias = smalls.tile([N, 1], F32, name="ebias")
            nc.vector.tensor_scalar_mul(ebias, mx, -TEMP_INV)
            for h in range(2):
                pdh = smalls.tile([N, 1], F32, name="pdh")
                nc.scalar.activation(p_t[:, h * mh:(h + 1) * mh], sps[h],
                                     AF.Exp, bias=ebias, scale=TEMP_INV,
                                     accum_out=pdh)
                if h == 0:
                    nc.vector.tensor_copy(out=pden, in_=pdh)
                else:
                    nc.vector.tensor_add(pden, pden, pdh)
            pinv = smalls.tile([N, 1], F32, name="pinv")
            nc.vector.reciprocal(pinv, pden)

            # ---------------- V natural + P^T interleaved ----------------
            v_nat = bigmat.tile([P, nmc, D2], BF16, tag="ktv", name="v_nat")
            vn2a = consts.tile([P, nmc], F32, name="vn2a")
            pt_sb = consts.tile([P, nmc, N], BF16, name="pt_sb")
            for q8 in range(8):
                wvq = wvqp.tile([P, nc_c0, 256], BF16, tag="wvq", name="wvq")
                nc.sync.dma_start(out=wvq, in_=wv_d[q8])
                d2s = q8 * 256
                for i in range(nmc):
                    vp = pnext([P, 256], F32, "vp")
                    for c in range(nc_c0):
                        nc.tensor.matmul(vp, xct[:, c, i * P:(i + 1) * P],
                                         wvq[:, c, :], start=(c == 0),
                                         stop=(c == nc_c0 - 1))
                    vni = v_nat[:, i, d2s:d2s + 256]
                    nc.vector.tensor_add(vni, vp,
                                         vb_bc[:, d2s:d2s + 256])
                    nc.vector.tensor_scalar_max(vni, vni, 0.0)
                    nc.vector.tensor_add(vni, vni,
                                         vc_bc[:, d2s:d2s + 256])
                    vsq = sqp.tile([P, 256], BF16, tag="sq", name="vsq")
                    vnq = smalls.tile([P, 1], F32, name="vnq")
                    nc.scalar.activation(vsq, vni, AF.Square, accum_out=vnq)
                    if q8 == 0:
                        nc.vector.tensor_copy(out=vn2a[:, i:i + 1], in_=vnq)
                    else:
                        nc.vector.tensor_add(vn2a[:, i:i + 1],
                                             vn2a[:, i:i + 1], vnq)
                    if q8 == 7:
                        rv = smalls.tile([P, 1], F32, name="rv")
                        nc.scalar.activation(rv, vn2a[:, i:i + 1], AF.Sqrt,
                                             bias=epsc)
                        nc.vector.reciprocal(rv, rv)
                        nc.vector.tensor_scalar(out=pt_sb[:, i, :],
                                                in0=pt_sb[:, i, :],
                                                scalar1=rv, scalar2=None,
                                                op0=ALU.mult)
                    if q8 == 0 and i == 2:
                        # P^T transposes (softmax finished during i=0,1)
                        ptp = pnext([P, nmc, N], BF16, "ptp")
                        for k in range(nmc):
                            nc.tensor.transpose(ptp[:, k, :],
                                                p_t[:, k * P:(k + 1) * P],
                                                ident[:N, :N])
                        nc.vector.tensor_copy(out=pt_sb, in_=ptp)

            # ---------------- WV natural [n, d2] ----------------
            wvb = nats.tile([N, D2], BF16, tag="nat2", name="wvb")
            for h in range(2):
                wvp = pnext([N, 1024], F32, "wvp")
                for i in range(nmc):
                    for (s, w) in _mtiles(1024):
                        nc.tensor.matmul(
                            wvp[:, s:s + w], pt_sb[:, i, :],
                            v_nat[:, i, h * 1024 + s:h * 1024 + s + w],
                            start=(i == 0), stop=(i == nmc - 1))
                nc.vector.tensor_scalar(out=wvb[:, h * 1024:(h + 1) * 1024],
                                        in0=wvp, scalar1=pinv, scalar2=None,
                                        op0=ALU.mult)
            wvT_ps = pnext([P, nc_d2, N], BF16, "wvT_ps")
            for c in range(nc_d2):
                nc.tensor.transpose(wvT_ps[:, c, :],
                                    wvb[:, c * P:(c + 1) * P], ident[:N, :N])
            wvT = consts.tile([P, nc_d2, N], BF16, name="wvT")
            nc.vector.tensor_copy(out=wvT, in_=wvT_ps)

            # ---------------- F natural [n, d0] ----------------
            fb_bc = bcast(fbv, N, D0, "b", "fb_bc")
            fc_bc = bcast(fcv, N, D0, "c", "fc_bc")
            fp = pnext([N, 1024], F32, "fp")
            for c in range(nc_d2):
                fw = strips.tile([P, D0], BF16, tag="strip", name="fw")
                eng = nc.sync if c % 2 == 0 else nc.scalar
                eng.dma_start(out=fw, in_=wf_d[c])
                for (s, w) in _mtiles(D0):
                    nc.tensor.matmul(fp[:, s:s + w], wvT[:, c, :],
                                     fw[:, s:s + w], start=(c == 0),
                                     stop=(c == nc_d2 - 1))
            fnat = nats.tile([N, D0], F32, tag="fnat", name="fnat")
            nc.vector.tensor_add(fnat, fp, fb_bc)
            nc.vector.tensor_scalar_max(fnat, fnat, 0.0)
            nc.vector.tensor_add(fnat, fnat, fc_bc)

            # ---------------- out = x + F (flat layout) ----------------
            f_scr = dscr.tile([N, D0], F32, name="f_scr")
            nc.sync.dma_start(out=f_scr, in_=fnat)
            fperm = consts.tile([P, D0 // 2], F32, name="fperm")
            nc.sync.dma_start(
                out=fperm,
                in_=bass.AP(tensor=f_scr.tensor, offset=f_scr.offset,
                            ap=[[D0, N], [D0 // 2, 2], [1, D0 // 2]]))
            for g in range(NFC):
                xo = xpool.tile([P, DQ, KK], F32, tag="x", name="xo")
                nc.scalar.dma_start(out=xo, in_=x_flat[:, g * FD:(g + 1) * FD])
                nc.vector.tensor_add(
                    xo, xo,
                    fperm[:, g * DQ:(g + 1) * DQ].unsqueeze(2)
                    .broadcast_to([P, DQ, KK]))
                nc.scalar.dma_start(out=out_flat[:, g * FD:(g + 1) * FD],
                                    in_=xo)

    nc.compile()
    return nc


_CACHED = {}
# test-harness hook: extra kwargs for run_bass_kernel_spmd (e.g. trace=True)
_RUN_KWARGS = {}


def _get_program(mp):
    key = ("nc", mp)
    if key not in _CACHED:
        _CACHED[key] = build_program(mp)
    return _CACHED[key]


def _fold(gamma, beta, mean, var, b):
    g = (np.asarray(gamma, np.float64)
         / np.sqrt(np.asarray(var, np.float64) + BN_EPS))
    bias = (g * np.asarray(b, np.float64)).astype(np.float32)
    cc = (np.asarray(beta, np.float64)
          - g * np.asarray(mean, np.float64)).astype(np.float32)
    return g.astype(np.float32), bias, cc


def kernel(**inputs):
    x = np.asarray(inputs["x"], dtype=np.float32).reshape(B, N, D0, KK)
    xc = np.asarray(inputs["x_context"], dtype=np.float32)
    nvalid = np.asarray(inputs["num_valid_context_items"]).reshape(B)
    nvalid = nvalid.astype(np.int64)

    mp = int(min(M, max(512, 256 * ((int(nvalid.max()) + 255) // 256))))

    gq, qbias, qcc = _fold(inputs["q_gamma"], inputs["q_beta"],
                           inputs["q_mean"], inputs["q_var"], inputs["q_b"])
    gk, kbias, kcc = _fold(inputs["k_gamma"], inputs["k_beta"],
                           inputs["k_mean"], inputs["k_var"], inputs["k_b"])
    gv, vbias, vcc = _fold(inputs["v_gamma"], inputs["v_beta"],
                           inputs["v_mean"], inputs["v_var"], inputs["v_b"])
    gf, fbias, fcc = _fold(inputs["f_gamma"], inputs["f_beta"],
                           inputs["f_mean"], inputs["f_var"], inputs["f_b"])

    bf = ml_dtypes.bfloat16
    # K strips: wk[j, p, c, q] = (gk*k_W)[j*128+q, c*128+p]
    kW = (np.asarray(inputs["k_W"], np.float32) * gk[:, None]).astype(bf)
    wk = np.ascontiguousarray(
        kW.reshape(D1 // P, P, C0 // P, P).transpose(0, 3, 2, 1))
    # Q strips: (gq*q_W/KK).T rows, [c, p, d1]
    qW = (np.asarray(inputs["q_W"], np.float32) * gq[:, None] / KK).astype(bf)
    wq = np.ascontiguousarray(qW.T.reshape(D0 // P, P, D1))
    # V eighth-blocks: wv[e, p, c, d] = (gv*v_W).T[c*128+p, e*256+d]
    vW = (np.asarray(inputs["v_W"], np.float32) * gv[:, None]).astype(bf)
    wv = np.ascontiguousarray(
        vW.T.reshape(C0 // P, P, 8, 256).transpose(2, 1, 0, 3))
    # F strips: (gf*f_W).T rows, [c, p, d0]
    fW = (np.asarray(inputs["f_W"], np.float32) * gf[:, None]).astype(bf)
    wf = np.ascontiguousarray(fW.T.reshape(D2 // P, P, D0))

    kcb = np.ascontiguousarray(kbias.reshape(D1 // P, P).T)
    kccf = np.ascontiguousarray(kcc.reshape(D1 // P, P).T)

    ar = np.arange(mp)
    in_maps = []
    for b in range(B):
        am = np.where(ar < nvalid[b], 0.0, NEG_MASK).astype(bf)
        in_maps.append({
            "x": np.ascontiguousarray(x[b]),
            "xct": np.ascontiguousarray(xc[b, :mp, :].T.astype(bf)),
            "wk": wk, "wq": wq, "wv": wv, "wf": wf,
            "kcb": kcb, "kcc": kccf,
            "qbv": qbias.astype(bf), "qcv": qcc.astype(bf),
            "vbv": vbias.astype(bf), "vcv": vcc.astype(bf),
            "fbv": fbias.astype(bf), "fcv": fcc.astype(bf),
            "amask": am,
        })

    nc = _get_program(mp)
    res = bass_utils.run_bass_kernel_spmd(nc, in_maps, core_ids=list(range(B)),
                                          **_RUN_KWARGS)
    _CACHED["last_results"] = res
    out = np.stack([r["out"] for r in res.results], axis=0)
    return out.reshape(B, N, D0, 7, 7).astype(np.float32)
